# revision 3
# baseline (speedup 1.0000x reference)
"""ChannelAttention Trainium2 Bass kernel.

Reference (per batch b, A = x[b] reshaped (H*W, C), H=W=64, C=512):
    scores = A^T @ At          (At = A with the 64x64 spatial grid transposed)
    P      = softmax(scores, axis=-1)   (rows on partition, cols on free dim)
    out    = A @ P
    y      = beta * out + x

Sharding: data-parallel over batch, 2 batches per core on 8 cores.

Numerics:
  - scores via hi/lo-split bf16 3-pass matmul (x = hi + lo, drop lo*lo):
    near-fp32 logits (abs err ~2e-4 on logits of scale ~200).
  - softmax in fp32 (max-subtracted exp on ACT, fp32 reductions on DVE).
  - out matmul in float32r (tf32-like, rel err ~1e-4; P is in [0,1]).
  - final beta*out + x in fp32 (beta folded into P columns).
"""
import os
import sys

sys.path.insert(0, "/opt/trn_rl_repo")

import numpy as np

import concourse.bacc as bacc
import concourse.bass as bass
import concourse.mybir as mybir
import concourse.tile as tile
from concourse import masks
from concourse.bass_utils import run_bass_kernel_spmd

B, H, W, C = 16, 64, 64, 512
N_CORES = 8
B_LOC = B // N_CORES          # batches per core
M = H * W                     # 4096 rows per batch
NCH = M // 128                # 32 row chunks
KCH = C // 128                # 4 channel chunks
F32 = mybir.dt.float32
F32R = mybir.dt.float32r
BF16 = mybir.dt.bfloat16
REPS = int(os.environ.get("KERNEL_REPS", "1"))

_cache = {}


def _build():
    nc = bacc.Bacc("TRN2", target_bir_lowering=False, debug=False,
                   num_devices=N_CORES)
    x_d = nc.dram_tensor("x", [B_LOC, H, W, C], F32, kind="ExternalInput")
    beta_d = nc.dram_tensor("beta", [C], F32, kind="ExternalInput")
    y_d = nc.dram_tensor("y", [B_LOC, H, W, C], F32, kind="ExternalOutput")

    # row-major (i j) view, chunked into 32 x [128, 512]
    a_src = x_d.ap().rearrange("b i j c -> b (i j) c").rearrange(
        "b (n p) c -> b n p c", p=128)
    y_dst = y_d.ap().rearrange("b i j c -> b (i j) c").rearrange(
        "b (n p) c -> b n p c", p=128)
    # spatially transposed view (j i): chunk n covers j in [2n, 2n+2), all i
    at_src = x_d.ap().rearrange("b i j c -> b j i c")

    with tile.TileContext(nc) as tc:
        with (
            tc.tile_pool(name="ld", bufs=3) as ld,
            tc.tile_pool(name="hilo", bufs=3) as hilo,
            tc.tile_pool(name="atr", bufs=1) as atr,
            tc.tile_pool(name="pp", bufs=2) as pp,
            tc.tile_pool(name="stats", bufs=4) as stats,
            tc.tile_pool(name="cst", bufs=1) as cst,
            tc.tile_pool(name="eps", bufs=3) as eps,
            tc.tile_pool(name="ps_s", bufs=1, space="PSUM") as ps_s,
            tc.tile_pool(name="ps_t", bufs=2, space="PSUM") as ps_t,
            tc.tile_pool(name="ps_o", bufs=2, space="PSUM") as ps_o,
        ):
            ident = cst.tile([128, 128], F32, tag="ident")
            masks.make_identity(nc, ident[:])
            beta_b = cst.tile([128, C], F32, tag="beta")
            nc.sync.dma_start(
                beta_b[:], beta_d.ap().unsqueeze(0).broadcast_to([128, C]))

            for rep in range(REPS):
                for b in range(B_LOC):
                    # ---- scores (3-pass bf16 hi/lo) + A^T transposes ----
                    ps = [ps_s.tile([128, C], F32, name=f"ps{k}", tag=f"ps{k}")
                          for k in range(KCH)]
                    a_t = atr.tile([128, KCH, M], F32R, tag="a_t")
                    for n in range(NCH):
                        a_f = ld.tile([128, C], F32, tag="a_f")
                        nc.sync.dma_start(a_f[:], a_src[b, n])
                        at_f = ld.tile([128, C], F32, tag="at_f")
                        for jj in range(2):
                            nc.sync.dma_start(
                                at_f[jj * 64:(jj + 1) * 64, :],
                                at_src[b, 2 * n + jj])

                        a_hi = hilo.tile([128, C], BF16, tag="a_hi")
                        nc.scalar.copy(a_hi[:], a_f[:])
                        at_hi = hilo.tile([128, C], BF16, tag="at_hi")
                        nc.scalar.copy(at_hi[:], at_f[:])
                        a_lo = hilo.tile([128, C], BF16, tag="a_lo")
                        nc.vector.tensor_sub(a_lo[:], a_f[:], a_hi[:])
                        at_lo = hilo.tile([128, C], BF16, tag="at_lo")
                        nc.vector.tensor_sub(at_lo[:], at_f[:], at_hi[:])

                        # A^T: 4 PE transposes (f32) into one PSUM bank,
                        # then one DVE copy (rounds to f32r)
                        tr = ps_t.tile([128, KCH, 128], F32, tag="tr")
                        for k in range(KCH):
                            nc.tensor.transpose(
                                tr[:, k, :], a_f[:, bass.ts(k, 128)], ident[:])
                        nc.vector.tensor_copy(
                            a_t[:, :, bass.ts(n, 128)], tr[:])

                        first, last = n == 0, n == NCH - 1
                        for k in range(KCH):
                            lhs_k = bass.ts(k, 128)
                            for pi, (lt, rt) in enumerate(
                                    ((a_hi, at_hi), (a_hi, at_lo), (a_lo, at_hi))):
                                nc.tensor.matmul(
                                    ps[k][:], lt[:, lhs_k], rt[:],
                                    start=(first and pi == 0),
                                    stop=(last and pi == 2))

                    # ---- softmax over free dim + beta fold -> f32r ----
                    p_r = [pp.tile([128, C], F32R, name=f"p_r{k}", tag=f"p_r{k}")
                           for k in range(KCH)]
                    for k in range(KCH):
                        negmx = stats.tile([128, 1], F32, tag="negmx")
                        nc.vector.reduce_max(
                            negmx[:], ps[k][:], axis=mybir.AxisListType.X,
                            negate=True)
                        p_f = pp.tile([128, C], F32, tag="p_f")
                        sm = stats.tile([128, 1], F32, tag="sm")
                        nc.scalar.activation(
                            p_f[:], ps[k][:], mybir.ActivationFunctionType.Exp,
                            bias=negmx[:], accum_out=sm[:])
                        rcp = stats.tile([128, 1], F32, tag="rcp")
                        nc.vector.reciprocal(rcp[:], sm[:])
                        # p_r = (p_f * rcp_row) * beta_col
                        nc.vector.scalar_tensor_tensor(
                            out=p_r[k][:], in0=p_f[:], scalar=rcp[:],
                            in1=beta_b[:], op0=mybir.AluOpType.mult,
                            op1=mybir.AluOpType.mult)

                    # ---- out = A @ P (f32r), epilogue add x ----
                    for n in range(NCH):
                        po = ps_o.tile([128, C], F32, tag="po")
                        for k in range(KCH):
                            nc.tensor.matmul(
                                po[:], a_t[:, k, bass.ts(n, 128)], p_r[k][:],
                                start=(k == 0), stop=(k == KCH - 1))
                        xe = eps.tile([128, C], F32, tag="xe")
                        nc.sync.dma_start(xe[:], a_src[b, n])
                        ob = eps.tile([128, C], F32, tag="ob")
                        nc.vector.tensor_add(ob[:], po[:], xe[:])
                        nc.sync.dma_start(y_dst[b, n], ob[:])
    nc.compile()
    return nc


def kernel(x: np.ndarray, beta: np.ndarray) -> np.ndarray:
    if "nc" not in _cache:
        _cache["nc"] = _build()
    nc = _cache["nc"]
    x = np.ascontiguousarray(x, dtype=np.float32)
    beta = np.ascontiguousarray(beta, dtype=np.float32)
    in_maps = [
        {"x": x[i * B_LOC:(i + 1) * B_LOC], "beta": beta}
        for i in range(N_CORES)
    ]
    res = run_bass_kernel_spmd(nc, in_maps, core_ids=list(range(N_CORES)))
    return np.concatenate([r["y"] for r in res.results], axis=0)


# revision 5
# speedup vs baseline: 1.0810x; 1.0810x over previous
"""ChannelAttention Trainium2 Bass kernel.

Reference (per batch b, A = x[b] reshaped (H*W, C), H=W=64, C=512):
    scores = A^T @ At          (At = A with the 64x64 spatial grid transposed)
    P      = softmax(scores, axis=-1)   (rows on partition, cols on free dim)
    out    = A @ P
    y      = beta * out + x

Sharding: data-parallel over batch, 2 batches per core on 8 cores.

Numerics:
  - scores via hi/lo-split bf16 3-pass matmul (x = hi + lo, drop lo*lo):
    near-fp32 logits (abs err ~2e-4 on logits of scale ~200).
  - softmax in fp32 (max-subtracted exp on ACT, fp32 reductions on DVE).
  - out matmul in float32r (tf32-like, rel err ~1e-4; P is in [0,1]).
  - final beta*out + x in fp32 (beta folded into P columns).
"""
import os
import sys

sys.path.insert(0, "/opt/trn_rl_repo")

import numpy as np

import concourse.bacc as bacc
import concourse.bass as bass
import concourse.mybir as mybir
import concourse.tile as tile
from concourse import masks
from concourse.bass_utils import run_bass_kernel_spmd

B, H, W, C = 16, 64, 64, 512
N_CORES = 8
B_LOC = B // N_CORES          # batches per core
M = H * W                     # 4096 rows per batch
NCH = M // 128                # 32 row chunks
KCH = C // 128                # 4 channel chunks
F32 = mybir.dt.float32
F32R = mybir.dt.float32r
BF16 = mybir.dt.bfloat16
REPS = int(os.environ.get("KERNEL_REPS", "1"))

_cache = {}


def _build():
    nc = bacc.Bacc("TRN2", target_bir_lowering=False, debug=False,
                   num_devices=N_CORES)
    x_d = nc.dram_tensor("x", [B_LOC, H, W, C], F32, kind="ExternalInput")
    beta_d = nc.dram_tensor("beta", [C], F32, kind="ExternalInput")
    y_d = nc.dram_tensor("y", [B_LOC, H, W, C], F32, kind="ExternalOutput")

    # row-major (i j) view, chunked into 32 x [128, 512]
    a_src = x_d.ap().rearrange("b i j c -> b (i j) c").rearrange(
        "b (n p) c -> b n p c", p=128)
    y_dst = y_d.ap().rearrange("b i j c -> b (i j) c").rearrange(
        "b (n p) c -> b n p c", p=128)
    # spatially transposed view (j i): chunk n covers j in [2n, 2n+2), all i
    at_src = x_d.ap().rearrange("b i j c -> b j i c")

    with tile.TileContext(nc) as tc:
        with (
            tc.tile_pool(name="ld", bufs=3) as ld,
            tc.tile_pool(name="hilo", bufs=3) as hilo,
            tc.tile_pool(name="atr", bufs=1) as atr,
            tc.tile_pool(name="pp", bufs=2) as pp,
            tc.tile_pool(name="stats", bufs=4) as stats,
            tc.tile_pool(name="cst", bufs=1) as cst,
            tc.tile_pool(name="eps", bufs=3) as eps,
            tc.tile_pool(name="ps_s", bufs=1, space="PSUM") as ps_s,
            tc.tile_pool(name="ps_t", bufs=2, space="PSUM") as ps_t,
            tc.tile_pool(name="ps_o", bufs=2, space="PSUM") as ps_o,
        ):
            ident = cst.tile([128, 128], F32, tag="ident")
            masks.make_identity(nc, ident[:])
            beta_b = cst.tile([128, C], F32, tag="beta")
            nc.sync.dma_start(
                beta_b[:], beta_d.ap().unsqueeze(0).broadcast_to([128, C]))

            for rep in range(REPS):
                for b in range(B_LOC):
                    # ---- scores (3-pass bf16 hi/lo) + A^T transposes ----
                    ps = [ps_s.tile([128, C], F32, name=f"ps{k}", tag=f"ps{k}")
                          for k in range(KCH)]
                    a_t = atr.tile([128, KCH, M], F32R, tag="a_t")
                    for n in range(NCH):
                        a_f = ld.tile([128, C], F32, tag="a_f")
                        nc.sync.dma_start(a_f[:], a_src[b, n])
                        at_f = ld.tile([128, C], F32, tag="at_f")
                        for jj in range(2):
                            nc.sync.dma_start(
                                at_f[jj * 64:(jj + 1) * 64, :],
                                at_src[b, 2 * n + jj])

                        a_hi = hilo.tile([128, C], BF16, tag="a_hi")
                        nc.scalar.copy(a_hi[:], a_f[:])
                        at_hi = hilo.tile([128, C], BF16, tag="at_hi")
                        nc.scalar.copy(at_hi[:], at_f[:])
                        a_lo = hilo.tile([128, C], BF16, tag="a_lo")
                        nc.vector.tensor_sub(a_lo[:], a_f[:], a_hi[:])
                        at_lo = hilo.tile([128, C], BF16, tag="at_lo")
                        nc.vector.tensor_sub(at_lo[:], at_f[:], at_hi[:])

                        # A^T: 4 PE transposes (f32) into one PSUM bank,
                        # then one DVE copy (rounds to f32r)
                        tr = ps_t.tile([128, KCH, 128], F32, tag="tr")
                        for k in range(KCH):
                            nc.tensor.transpose(
                                tr[:, k, :], a_f[:, bass.ts(k, 128)], ident[:])
                        nc.vector.tensor_copy(
                            a_t[:, :, bass.ts(n, 128)], tr[:])

                        first, last = n == 0, n == NCH - 1
                        for k in range(KCH):
                            lhs_k = bass.ts(k, 128)
                            for pi, (lt, rt) in enumerate(
                                    ((a_hi, at_hi), (a_hi, at_lo), (a_lo, at_hi))):
                                nc.tensor.matmul(
                                    ps[k][:], lt[:, lhs_k], rt[:],
                                    start=(first and pi == 0),
                                    stop=(last and pi == 2))

                    # ---- softmax over free dim + beta fold -> f32r ----
                    p_r = [pp.tile([128, C], F32R, name=f"p_r{k}", tag=f"p_r{k}")
                           for k in range(KCH)]
                    for k in range(KCH):
                        negmx = stats.tile([128, 1], F32, tag="negmx")
                        nc.vector.reduce_max(
                            negmx[:], ps[k][:], axis=mybir.AxisListType.X,
                            negate=True)
                        p_f = pp.tile([128, C], F32, tag="p_f")
                        sm = stats.tile([128, 1], F32, tag="sm")
                        nc.scalar.activation(
                            p_f[:], ps[k][:], mybir.ActivationFunctionType.Exp,
                            bias=negmx[:], accum_out=sm[:])
                        rcp = stats.tile([128, 1], F32, tag="rcp")
                        nc.vector.reciprocal(rcp[:], sm[:])
                        # p_r = (p_f * rcp_row) * beta_col
                        nc.vector.scalar_tensor_tensor(
                            out=p_r[k][:], in0=p_f[:], scalar=rcp[:],
                            in1=beta_b[:], op0=mybir.AluOpType.mult,
                            op1=mybir.AluOpType.mult)

                    # ---- out = A @ P (f32r), epilogue add x ----
                    for n in range(NCH):
                        po = ps_o.tile([128, C], F32, tag="po")
                        for k in range(KCH):
                            nc.tensor.matmul(
                                po[:], a_t[:, k, bass.ts(n, 128)], p_r[k][:],
                                start=(k == 0), stop=(k == KCH - 1))
                        xe = eps.tile([128, C], F32, tag="xe")
                        nc.sync.dma_start(xe[:], a_src[b, n])
                        ob = eps.tile([128, C], F32, tag="ob")
                        nc.vector.tensor_add(ob[:], po[:], xe[:])
                        nc.sync.dma_start(y_dst[b, n], ob[:])
    nc.compile()
    return nc


def _build_runner():
    """Build the Bass module once and wrap it in a cached jitted shard_map
    callable (mirrors concourse.bass2jax.run_bass_via_pjrt's multi-core
    branch, but without per-call retracing)."""
    import jax
    from jax.experimental.shard_map import shard_map
    from jax.sharding import Mesh, PartitionSpec

    from concourse.bass2jax import (
        _bass_exec_p,
        install_neuronx_cc_hook,
        partition_id_tensor,
    )

    nc = _build()
    install_neuronx_cc_hook()

    import concourse.mybir as _mb

    in_names = ["x", "beta"]
    out_names = ["y"]
    out_avals = [jax.core.ShapedArray((B_LOC, H, W, C), np.float32)]
    all_names = in_names + out_names
    partition_name = (
        nc.partition_id_tensor.name if nc.partition_id_tensor else None)
    if partition_name is not None:
        all_names.append(partition_name)

    def _body(*args):
        operands = list(args)
        if partition_name is not None:
            operands.append(partition_id_tensor())
        outs = _bass_exec_p.bind(
            *operands,
            out_avals=tuple(out_avals),
            in_names=tuple(all_names),
            out_names=tuple(out_names),
            lowering_input_output_aliases=(),
            sim_require_finite=True,
            sim_require_nnan=True,
            nc=nc,
        )
        return tuple(outs)

    devices = jax.devices()[:N_CORES]
    mesh = Mesh(np.asarray(devices), ("core",))
    n_in = len(in_names)
    sharded = jax.jit(
        shard_map(
            _body, mesh=mesh,
            in_specs=(PartitionSpec("core"),) * (n_in + 1),
            out_specs=(PartitionSpec("core"),),
            check_rep=False,
        ),
        donate_argnums=(n_in,),
        keep_unused=True,
    )
    return sharded


def _run(x: np.ndarray, beta: np.ndarray) -> np.ndarray:
    if "fn" not in _cache:
        _cache["fn"] = _build_runner()
    fn = _cache["fn"]
    beta_rep = np.ascontiguousarray(
        np.broadcast_to(beta, (N_CORES, C))).reshape(N_CORES * C)
    zeros = np.zeros((B, H, W, C), np.float32)
    (y,) = fn(x, beta_rep, zeros)
    return np.asarray(y)


def kernel(x: np.ndarray, beta: np.ndarray) -> np.ndarray:
    x = np.ascontiguousarray(x, dtype=np.float32)
    beta = np.ascontiguousarray(beta, dtype=np.float32)
    return _run(x, beta)


# revision 6
# speedup vs baseline: 1.1250x; 1.0407x over previous
"""ChannelAttention Trainium2 Bass kernel.

Reference (per batch b, A = x[b] reshaped (H*W, C), H=W=64, C=512):
    scores = A^T @ At          (At = A with the 64x64 spatial grid transposed)
    P      = softmax(scores, axis=-1)   (rows on partition, cols on free dim)
    out    = A @ P
    y      = beta * out + x

Sharding: data-parallel over batch, 2 batches per core on 8 cores.

Numerics:
  - scores via hi/lo-split bf16 3-pass matmul (x = hi + lo, drop lo*lo):
    near-fp32 logits (abs err ~2e-4 on logits of scale ~200).
  - softmax in fp32 (max-subtracted exp on ACT, fp32 reductions on DVE).
  - out matmul in float32r (tf32-like, rel err ~1e-4; P is in [0,1]).
  - final beta*out + x in fp32 (beta folded into P columns).
"""
import os
import sys

sys.path.insert(0, "/opt/trn_rl_repo")

import numpy as np

import concourse.bacc as bacc
import concourse.bass as bass
import concourse.mybir as mybir
import concourse.tile as tile
from concourse import masks
from concourse.bass_utils import run_bass_kernel_spmd

B, H, W, C = 16, 64, 64, 512
N_CORES = 8
B_LOC = B // N_CORES          # batches per core
M = H * W                     # 4096 rows per batch
NCH = M // 128                # 32 row chunks
KCH = C // 128                # 4 channel chunks
F32 = mybir.dt.float32
F32R = mybir.dt.float32r
BF16 = mybir.dt.bfloat16
REPS = int(os.environ.get("KERNEL_REPS", "1"))

_cache = {}


def _build():
    nc = bacc.Bacc("TRN2", target_bir_lowering=False, debug=False,
                   num_devices=N_CORES)
    x_d = nc.dram_tensor("x", [B_LOC, H, W, C], F32, kind="ExternalInput")
    beta_d = nc.dram_tensor("beta", [C], F32, kind="ExternalInput")
    y_d = nc.dram_tensor("y", [B_LOC, H, W, C], F32, kind="ExternalOutput")

    # row-major (i j) view, chunked into 32 x [128, 512]
    a_src = x_d.ap().rearrange("b i j c -> b (i j) c").rearrange(
        "b (n p) c -> b n p c", p=128)
    y_dst = y_d.ap().rearrange("b i j c -> b (i j) c").rearrange(
        "b (n p) c -> b n p c", p=128)
    # spatially transposed view (j i): chunk n covers j in [2n, 2n+2), all i
    at_src = x_d.ap().rearrange("b i j c -> b j i c")

    with tile.TileContext(nc) as tc:
        with (
            tc.tile_pool(name="ld", bufs=3) as ld,
            tc.tile_pool(name="hilo", bufs=3) as hilo,
            tc.tile_pool(name="atr", bufs=1) as atr,
            tc.tile_pool(name="pp", bufs=2) as pp,
            tc.tile_pool(name="stats", bufs=4) as stats,
            tc.tile_pool(name="cst", bufs=1) as cst,
            tc.tile_pool(name="eps", bufs=3) as eps,
            tc.tile_pool(name="ps_s", bufs=1, space="PSUM") as ps_s,
            tc.tile_pool(name="ps_t", bufs=2, space="PSUM") as ps_t,
            tc.tile_pool(name="ps_o", bufs=2, space="PSUM") as ps_o,
        ):
            ident = cst.tile([128, 128], F32, tag="ident")
            masks.make_identity(nc, ident[:])
            beta_b = cst.tile([128, C], F32, tag="beta")
            nc.sync.dma_start(
                beta_b[:], beta_d.ap().unsqueeze(0).broadcast_to([128, C]))

            for rep in range(REPS):
                for b in range(B_LOC):
                    # ---- scores (3-pass bf16 hi/lo), upper-triangular
                    # blocks only (scores is symmetric), + A^T transposes ----
                    ps = [ps_s.tile([128, C - 128 * k], F32,
                                    name=f"ps{k}", tag=f"ps{k}")
                          for k in range(KCH)]
                    a_t = atr.tile([128, KCH, M], F32R, tag="a_t")
                    for n in range(NCH):
                        a_f = ld.tile([128, C], F32, tag="a_f")
                        nc.sync.dma_start(a_f[:], a_src[b, n])
                        at_f = ld.tile([128, C], F32, tag="at_f")
                        for jj in range(2):
                            nc.sync.dma_start(
                                at_f[jj * 64:(jj + 1) * 64, :],
                                at_src[b, 2 * n + jj])

                        a_hi = hilo.tile([128, C], BF16, tag="a_hi")
                        nc.scalar.copy(a_hi[:], a_f[:])
                        at_hi = hilo.tile([128, C], BF16, tag="at_hi")
                        nc.scalar.copy(at_hi[:], at_f[:])
                        a_lo = hilo.tile([128, C], BF16, tag="a_lo")
                        nc.vector.tensor_sub(a_lo[:], a_f[:], a_hi[:])
                        at_lo = hilo.tile([128, C], BF16, tag="at_lo")
                        nc.vector.tensor_sub(at_lo[:], at_f[:], at_hi[:])

                        # A^T: 4 PE transposes (f32) into one PSUM bank,
                        # then one DVE copy (rounds to f32r)
                        tr = ps_t.tile([128, KCH, 128], F32, tag="tr")
                        for k in range(KCH):
                            nc.tensor.transpose(
                                tr[:, k, :], a_f[:, bass.ts(k, 128)], ident[:])
                        nc.vector.tensor_copy(
                            a_t[:, :, bass.ts(n, 128)], tr[:])

                        first, last = n == 0, n == NCH - 1
                        for k in range(KCH):
                            lhs_k = bass.ts(k, 128)
                            for pi, (lt, rt) in enumerate(
                                    ((a_hi, at_hi), (a_hi, at_lo), (a_lo, at_hi))):
                                nc.tensor.matmul(
                                    ps[k][:], lt[:, lhs_k], rt[:, 128 * k:],
                                    start=(first and pi == 0),
                                    stop=(last and pi == 2))

                    # ---- assemble full score rows in SBUF:
                    # direct (upper) parts + transposed (lower) parts ----
                    sc = [pp.tile([128, C], F32, name=f"sc{k}", tag=f"sc{k}")
                          for k in range(KCH)]
                    for k in range(KCH):
                        nc.vector.tensor_copy(sc[k][:, 128 * k:], ps[k][:])
                    for k in range(1, KCH):
                        # lower blocks (k, l<k) = transpose of sc[l] block k
                        tr = ps_t.tile([128, KCH, 128], F32, tag="tr")
                        for lb in range(k):
                            nc.tensor.transpose(
                                tr[:, lb, :], sc[lb][:, bass.ts(k, 128)],
                                ident[:])
                        nc.vector.tensor_copy(sc[k][:, :128 * k],
                                              tr[:, :k, :])

                    # ---- softmax over free dim + beta fold -> f32r ----
                    p_r = [pp.tile([128, C], F32R, name=f"p_r{k}", tag=f"p_r{k}")
                           for k in range(KCH)]
                    for k in range(KCH):
                        negmx = stats.tile([128, 1], F32, tag="negmx")
                        nc.vector.reduce_max(
                            negmx[:], sc[k][:], axis=mybir.AxisListType.X,
                            negate=True)
                        p_f = pp.tile([128, C], F32, tag="p_f")
                        sm = stats.tile([128, 1], F32, tag="sm")
                        nc.scalar.activation(
                            p_f[:], sc[k][:], mybir.ActivationFunctionType.Exp,
                            bias=negmx[:], accum_out=sm[:])
                        rcp = stats.tile([128, 1], F32, tag="rcp")
                        nc.vector.reciprocal(rcp[:], sm[:])
                        # p_r = (p_f * rcp_row) * beta_col
                        nc.vector.scalar_tensor_tensor(
                            out=p_r[k][:], in0=p_f[:], scalar=rcp[:],
                            in1=beta_b[:], op0=mybir.AluOpType.mult,
                            op1=mybir.AluOpType.mult)

                    # ---- out = A @ P (f32r), epilogue add x ----
                    for n in range(NCH):
                        po = ps_o.tile([128, C], F32, tag="po")
                        for k in range(KCH):
                            nc.tensor.matmul(
                                po[:], a_t[:, k, bass.ts(n, 128)], p_r[k][:],
                                start=(k == 0), stop=(k == KCH - 1))
                        xe = eps.tile([128, C], F32, tag="xe")
                        nc.sync.dma_start(xe[:], a_src[b, n])
                        ob = eps.tile([128, C], F32, tag="ob")
                        nc.vector.tensor_add(ob[:], po[:], xe[:])
                        nc.sync.dma_start(y_dst[b, n], ob[:])
    nc.compile()
    return nc


def _build_runner():
    """Build the Bass module once and wrap it in a cached jitted shard_map
    callable (mirrors concourse.bass2jax.run_bass_via_pjrt's multi-core
    branch, but without per-call retracing)."""
    import jax
    from jax.experimental.shard_map import shard_map
    from jax.sharding import Mesh, PartitionSpec

    from concourse.bass2jax import (
        _bass_exec_p,
        install_neuronx_cc_hook,
        partition_id_tensor,
    )

    nc = _build()
    install_neuronx_cc_hook()

    import concourse.mybir as _mb

    in_names = ["x", "beta"]
    out_names = ["y"]
    out_avals = [jax.core.ShapedArray((B_LOC, H, W, C), np.float32)]
    all_names = in_names + out_names
    partition_name = (
        nc.partition_id_tensor.name if nc.partition_id_tensor else None)
    if partition_name is not None:
        all_names.append(partition_name)

    def _body(*args):
        operands = list(args)
        if partition_name is not None:
            operands.append(partition_id_tensor())
        outs = _bass_exec_p.bind(
            *operands,
            out_avals=tuple(out_avals),
            in_names=tuple(all_names),
            out_names=tuple(out_names),
            lowering_input_output_aliases=(),
            sim_require_finite=True,
            sim_require_nnan=True,
            nc=nc,
        )
        return tuple(outs)

    devices = jax.devices()[:N_CORES]
    mesh = Mesh(np.asarray(devices), ("core",))
    n_in = len(in_names)
    sharded = jax.jit(
        shard_map(
            _body, mesh=mesh,
            in_specs=(PartitionSpec("core"),) * (n_in + 1),
            out_specs=(PartitionSpec("core"),),
            check_rep=False,
        ),
        donate_argnums=(n_in,),
        keep_unused=True,
    )
    return sharded


def _run(x: np.ndarray, beta: np.ndarray) -> np.ndarray:
    if "fn" not in _cache:
        _cache["fn"] = _build_runner()
    fn = _cache["fn"]
    beta_rep = np.ascontiguousarray(
        np.broadcast_to(beta, (N_CORES, C))).reshape(N_CORES * C)
    zeros = np.zeros((B, H, W, C), np.float32)
    (y,) = fn(x, beta_rep, zeros)
    return np.asarray(y)


def kernel(x: np.ndarray, beta: np.ndarray) -> np.ndarray:
    x = np.ascontiguousarray(x, dtype=np.float32)
    beta = np.ascontiguousarray(beta, dtype=np.float32)
    return _run(x, beta)
